# revision 7
# baseline (speedup 1.0000x reference)
"""ABCDense (ShiftedSteSign 3-estimator binary dense) Trainium2 kernel.

Math:
    xq   = sign(x)                      [N, D]   (+1 for x >= 0)
    beta = mean(|x|, axis=-1)           [N]
    out  = sum_e (xq @ sign(k_e)) * (beta[:,None] * a_e[None,:])

Folded form used here (column scaling commutes with the GEMM):
    W    = sum_e sign(k_e) * (a_e / D)[None,:]        [D, U]  (bf16)
    out  = sum_abs_x[:,None] * (xq @ W)

Distribution: pure data-parallel over the N=32768 token axis across 8
cores (4096 rows each); k/a replicated (no collectives: any cross-core
sync point adds the cores' launch skew to the measured span).

Per-core structure, software-pipelined with lookahead L (so PSUM
evacuations never queue behind future tiles' work in the strict-FIFO
ACT/DVE queues):
  prep(t):  load x f32 (d-halves) -> DVE |x|=max(-x,x) fused row-sum
            (beta) -> ACT Sign f32->bf16 natural -> DMA xbar transpose
            (SBUF->SBUF bf16) into xqT [d-part, c, tok], per u-half
  body(t):  16 bf16 matmuls (lhsT=xqT chunk stationary, W streaming)
            -> ACT PSUM evac (Copy) with fused per-partition beta scale
            -> out DMA
The PE does nothing but the 512 N=512 matmuls; the +-1 GEMM is exact
in bf16; PSUM accumulates exact integers.
k DMAs are split into 512KB pieces so they spread across all 16 DMA
queues (a single 2MB piece pins one ~23GB/s queue for ~85us); W is
prepared in u-halves so the first matmuls only wait for half of it.
"""

import numpy as np

import concourse.mybir as mybir
from concourse import bacc, tile
from concourse.bass_utils import run_bass_kernel_spmd

F32 = mybir.dt.float32
BF16 = mybir.dt.bfloat16
AF = mybir.ActivationFunctionType
ALU = mybir.AluOpType

N, D, U, E = 32768, 1024, 1024, 3
NCORES = 8
NS = N // NCORES            # 4096 rows per core
P = 128                     # partitions
DC = D // P                 # 8 d-chunks
NT = NS // P                # 32 n-tiles of 128 tokens
UH = 512                    # u half (one PSUM bank per matmul)
DH = D // 2                 # d half (for split x loads / sign / transpose)
LOOK = 6                    # software pipeline depth (tiles)


def build_nc():
    nc = bacc.Bacc(
        "TRN2",
        target_bir_lowering=False,
        debug=False,
        num_devices=NCORES,
    )

    # --- DRAM parameters (per-core shapes) ---
    x_d = nc.dram_tensor("x", [NS, D], F32, kind="ExternalInput")
    ks = [nc.dram_tensor(f"k{e}", [D, U], F32, kind="ExternalInput") for e in range(E)]
    as_ = [nc.dram_tensor(f"a{e}", [1, U], F32, kind="ExternalInput") for e in range(E)]
    out_d = nc.dram_tensor("out", [NS, U], BF16, kind="ExternalOutput")

    with tile.TileContext(nc) as tc:
        with (
            tc.tile_pool(name="const", bufs=1) as const,
            tc.tile_pool(name="kstage", bufs=3) as kpool,
            tc.tile_pool(name="xin", bufs=LOOK + 2) as xpool,
            tc.tile_pool(name="scr", bufs=3) as scrpool,
            tc.tile_pool(name="xqn", bufs=4) as xqnpool,
            tc.tile_pool(name="xqt", bufs=LOOK + 2) as xqtpool,
            tc.tile_pool(name="osb", bufs=3) as opool,
            tc.tile_pool(name="psM", bufs=4, space="PSUM") as psumM,
        ):
            beta_cols = const.tile([P, NT], F32)

            # ---------- k loads first: small pieces across all queues ----------
            # per (e, u-half): 4 pieces of [128, 2, 512] f32 (512KB each)
            kstage = {}
            for uh in range(2):
                us = uh * UH
                for e in range(E):
                    ke = kpool.tile([P, DC, UH], F32, tag="ke")
                    for cc in range(4):
                        nc.sync.dma_start(
                            out=ke[:, cc * 2:(cc + 1) * 2, :],
                            in_=ks[e][cc * 2 * P:(cc + 1) * 2 * P, us:us + UH]
                            .rearrange("(c p) u -> p c u", p=P),
                        )
                    kstage[(uh, e)] = ke

            # a_e / D as bf16, broadcast across partitions
            a_bcast = []
            for e in range(E):
                a_f = const.tile([1, U], F32, tag=f"a_f{e}")
                nc.sync.dma_start(out=a_f[0:1, :], in_=as_[e][:, :])
                a_b = const.tile([1, U], BF16, tag=f"a_b{e}")
                nc.vector.tensor_scalar(
                    a_b[0:1, :], a_f[0:1, :], 1.0 / D, None, op0=ALU.mult
                )
                a_full = const.tile([P, U], BF16, tag=f"a_full{e}")
                nc.gpsimd.partition_broadcast(a_full[:], a_b[0:1, :])
                a_bcast.append(a_full)

            # ---------- W = sum_e sign(k_e) * a_e / D  (bf16, [d-part, c, u]) ----------
            W = const.tile([P, DC, U], BF16)
            for uh in range(2):
                us = uh * UH
                for e in range(E):
                    ke = kstage[(uh, e)]
                    s_e = kpool.tile([P, DC, UH], BF16, tag="se")
                    for q in range(4):
                        nc.scalar.activation(
                            s_e[:, q * 2:(q + 1) * 2, :],
                            ke[:, q * 2:(q + 1) * 2, :],
                            AF.Sign,
                        )
                    for c in range(DC):
                        if e == 0:
                            nc.vector.tensor_tensor(
                                W[:, c, us:us + UH], s_e[:, c, :],
                                a_bcast[e][:, us:us + UH], op=ALU.mult
                            )
                        else:
                            tmp = kpool.tile([P, UH], BF16, tag="tmp")
                            nc.vector.tensor_tensor(
                                tmp[:], s_e[:, c, :],
                                a_bcast[e][:, us:us + UH], op=ALU.mult
                            )
                            nc.vector.tensor_tensor(
                                W[:, c, us:us + UH], W[:, c, us:us + UH],
                                tmp[:], op=ALU.add
                            )

            # ---------- software-pipelined main loop ----------
            def prep(t):
                x_t = xpool.tile([P, D], F32, tag="xt")
                for dh in range(2):
                    nc.sync.dma_start(
                        out=x_t[:, dh * DH:(dh + 1) * DH],
                        in_=x_d[t * P:(t + 1) * P, dh * DH:(dh + 1) * DH],
                    )
                # beta: |x|=max(-x,x) with fused row-sum on DVE (raw sum;
                # the 1/D of beta is folded into W)
                scratch = scrpool.tile([P, D], F32, tag="scratch")
                nc.vector.scalar_tensor_tensor(
                    scratch[:], x_t[:], -1.0, x_t[:],
                    op0=ALU.mult, op1=ALU.max,
                    accum_out=beta_cols[:, t:t + 1],
                )
                # sign (f32 -> bf16, natural) + xbar transpose, per d-half
                xq_n = xqnpool.tile([P, D], BF16, tag="xqn")
                xqT = xqtpool.tile([P, DC, P], BF16, tag="xqT")
                for dh in range(2):
                    nc.scalar.activation(
                        xq_n[:, dh * DH:(dh + 1) * DH],
                        x_t[:, dh * DH:(dh + 1) * DH], AF.Sign,
                    )
                    nc.sync.dma_start_transpose(
                        xqT[:, dh * 4:(dh + 1) * 4, :],
                        xq_n[:, dh * DH:(dh + 1) * DH],
                    )
                return xqT

            xqTs = {}
            for t in range(LOOK):
                xqTs[t] = prep(t)

            for t in range(NT):
                xqT = xqTs.pop(t)
                ps0 = psumM.tile([P, UH], F32, tag="ps0")
                ps1 = psumM.tile([P, UH], F32, tag="ps1")
                ps = [ps0, ps1]
                for c in range(DC):
                    for h in range(2):
                        nc.tensor.matmul(
                            ps[h][:],
                            xqT[:, c, :],
                            W[:, c, h * UH:(h + 1) * UH],
                            start=(c == 0), stop=(c == DC - 1),
                        )
                # PSUM evacuation on ACT with fused per-partition beta
                # scale (bf16 out)
                osb = opool.tile([P, U], BF16, tag="osb")
                bcol = beta_cols[:, t:t + 1]
                for h in range(2):
                    nc.scalar.activation(
                        osb[:, h * UH:(h + 1) * UH], ps[h][:],
                        AF.Copy, scale=bcol,
                    )
                nc.sync.dma_start(out=out_d[t * P:(t + 1) * P, :], in_=osb[:])
                if t + LOOK < NT:
                    xqTs[t + LOOK] = prep(t + LOOK)

    nc.compile()
    return nc


_CACHE = {}


def _get_nc():
    if "nc" not in _CACHE:
        _CACHE["nc"] = build_nc()
    return _CACHE["nc"]


def make_in_maps(x, k0, k1, k2, a0, a1, a2):
    x = np.ascontiguousarray(x, dtype=np.float32)
    ks = [np.ascontiguousarray(k, dtype=np.float32) for k in (k0, k1, k2)]
    as_ = [np.ascontiguousarray(a, dtype=np.float32).reshape(1, U) for a in (a0, a1, a2)]
    in_maps = []
    for i in range(NCORES):
        shard = np.ascontiguousarray(x[i * NS:(i + 1) * NS])
        in_maps.append({
            "x": shard,
            **{f"k{e}": ks[e] for e in range(E)},
            **{f"a{e}": as_[e] for e in range(E)},
        })
    return in_maps


def run_sharded(x, k0, k1, k2, a0, a1, a2, trace=False, **kw):
    nc = _get_nc()
    in_maps = make_in_maps(x, k0, k1, k2, a0, a1, a2)
    res = run_bass_kernel_spmd(nc, in_maps, list(range(NCORES)), trace=trace, **kw)
    out = np.concatenate(
        [np.asarray(res.results[i]["out"]).astype(np.float32) for i in range(NCORES)],
        axis=0,
    )
    return out, res


def kernel(x, k0, k1, k2, a0, a1, a2):
    out, _ = run_sharded(x, k0, k1, k2, a0, a1, a2, trace=False)
    return out


# revision 9
# speedup vs baseline: 1.2044x; 1.2044x over previous
"""ABCDense (ShiftedSteSign 3-estimator binary dense) Trainium2 kernel.

Math:
    xq   = sign(x)                      [N, D]   (+1 for x >= 0)
    beta = mean(|x|, axis=-1)           [N]
    out  = sum_e (xq @ sign(k_e)) * (beta[:,None] * a_e[None,:])

Folded form used here (column scaling commutes with the GEMM):
    W    = sum_e sign(k_e) * (a_e / D)[None,:]        [D, U]  (bf16)
    out  = sum_abs_x[:,None] * (xq @ W)

Distribution: pure data-parallel over the N=32768 token axis across 8
cores (4096 rows each); k/a replicated (no collectives: any cross-core
sync point adds the cores' launch skew to the measured span).

Queue decoupling (the point of this version): the two HWDGE rings are
split by role -- Sync issues only x loads / k loads / out stores, the
Scalar (ACT) sequencer issues the xbar transposes right after the sign
it depends on (same FIFO, so no cross-engine wait can block the Sync
ring's head).  PSUM evacuation is split DVE(h0)/ACT(h1) so neither
vector FIFO couples next-tile prep to current-tile matmuls.

Per-tile: x f32 -> DVE |x| fused row-sum (beta) -> ACT Sign f32->bf16
natural -> ACT-issued xbar transpose (SBUF->SBUF bf16) -> 16 bf16
matmuls (h-outer: 8 into each PSUM bank) -> split evac with fused
per-partition beta scale -> out DMA.  The PE does nothing but the 512
N=512 matmuls; the +-1 GEMM is exact in bf16.
"""

import numpy as np

import concourse.mybir as mybir
from concourse import bacc, tile
from concourse.bass_utils import run_bass_kernel_spmd

F32 = mybir.dt.float32
BF16 = mybir.dt.bfloat16
AF = mybir.ActivationFunctionType
ALU = mybir.AluOpType

N, D, U, E = 32768, 1024, 1024, 3
NCORES = 8
NS = N // NCORES            # 4096 rows per core
P = 128                     # partitions
DC = D // P                 # 8 d-chunks
NT = NS // P                # 32 n-tiles of 128 tokens
UH = 512                    # u half (one PSUM bank per matmul)
LOOK = 8                    # software pipeline depth (tiles)


def build_nc():
    nc = bacc.Bacc(
        "TRN2",
        target_bir_lowering=False,
        debug=False,
        num_devices=NCORES,
    )

    # --- DRAM parameters (per-core shapes) ---
    x_d = nc.dram_tensor("x", [NS, D], F32, kind="ExternalInput")
    ks = [nc.dram_tensor(f"k{e}", [D, U], F32, kind="ExternalInput") for e in range(E)]
    as_ = [nc.dram_tensor(f"a{e}", [1, U], F32, kind="ExternalInput") for e in range(E)]
    out_d = nc.dram_tensor("out", [NS, U], BF16, kind="ExternalOutput")

    with tile.TileContext(nc) as tc:
        with (
            tc.tile_pool(name="const", bufs=1) as const,
            tc.tile_pool(name="kstage", bufs=2) as kpool,
            tc.tile_pool(name="xin", bufs=LOOK + 1) as xpool,
            tc.tile_pool(name="scr", bufs=3) as scrpool,
            tc.tile_pool(name="xqn", bufs=3) as xqnpool,
            tc.tile_pool(name="xqt", bufs=LOOK + 2) as xqtpool,
            tc.tile_pool(name="osb", bufs=3) as opool,
            tc.tile_pool(name="psM", bufs=8, space="PSUM") as psumM,
        ):
            beta_cols = const.tile([P, NT], F32)

            # a loads first (tiny; the folds need them early)
            a_fs = []
            for e in range(E):
                a_f = const.tile([1, U], F32, tag=f"a_f{e}")
                nc.sync.dma_start(out=a_f[0:1, :], in_=as_[e][:, :])
                a_fs.append(a_f)

            # k loads: one 2MB dma_start per estimator (splits across all
            # 16 SDMA engine slots internally; >=1MiB for full rate)
            kes = []
            for e in range(E):
                ke = kpool.tile([P, DC, U], F32, tag="ke")
                nc.sync.dma_start(
                    out=ke[:],
                    in_=ks[e][:, :].rearrange("(c p) u -> p c u", p=P),
                )
                kes.append(ke)

            a_bcast = []
            for e in range(E):
                a_b = const.tile([1, U], BF16, tag=f"a_b{e}")
                nc.vector.tensor_scalar(
                    a_b[0:1, :], a_fs[e][0:1, :], 1.0 / D, None, op0=ALU.mult
                )
                a_full = const.tile([P, U], BF16, tag=f"a_full{e}")
                nc.gpsimd.partition_broadcast(a_full[:], a_b[0:1, :])
                a_bcast.append(a_full)

            # ---------- W = sum_e sign(k_e) * a_e / D  (bf16, [d-part, c, u]) ----------
            W = const.tile([P, DC, U], BF16)
            for e in range(E):
                ke = kes[e]
                for q in range(4):
                    s_q = kpool.tile([P, 2, U], BF16, tag="se")
                    nc.scalar.activation(
                        s_q[:], ke[:, q * 2:(q + 1) * 2, :], AF.Sign,
                    )
                    for cc in range(2):
                        c = q * 2 + cc
                        if e == 0:
                            nc.vector.tensor_tensor(
                                W[:, c, :], s_q[:, cc, :], a_bcast[e][:],
                                op=ALU.mult
                            )
                        else:
                            tmp = kpool.tile([P, U], BF16, tag="tmp")
                            nc.vector.tensor_tensor(
                                tmp[:], s_q[:, cc, :], a_bcast[e][:],
                                op=ALU.mult
                            )
                            nc.vector.tensor_tensor(
                                W[:, c, :], W[:, c, :], tmp[:], op=ALU.add
                            )

            # ---------- software-pipelined main loop ----------
            def prep(t):
                x_t = xpool.tile([P, D], F32, tag="xt")
                nc.sync.dma_start(out=x_t[:], in_=x_d[t * P:(t + 1) * P, :])
                # beta: |x|=max(-x,x) with fused row-sum on DVE (raw sum;
                # the 1/D of beta is folded into W)
                scratch = scrpool.tile([P, D], F32, tag="scratch")
                nc.vector.scalar_tensor_tensor(
                    scratch[:], x_t[:], -1.0, x_t[:],
                    op0=ALU.mult, op1=ALU.max,
                    accum_out=beta_cols[:, t:t + 1],
                )
                # sign (f32 -> bf16, natural) on ACT, then the xbar
                # transpose issued from the ACT sequencer (same FIFO)
                xq_n = xqnpool.tile([P, D], BF16, tag="xqn")
                nc.scalar.activation(xq_n[:], x_t[:], AF.Sign)
                xqT = xqtpool.tile([P, DC, P], BF16, tag="xqT")
                nc.scalar.dma_start_transpose(xqT[:], xq_n[:])
                return xqT

            xqTs = {}
            for t in range(LOOK):
                xqTs[t] = prep(t)

            for t in range(NT):
                xqT = xqTs.pop(t)
                ps = []
                for h in range(2):
                    p_h = psumM.tile([P, UH], F32, tag="ps")
                    ps.append(p_h)
                    for c in range(DC):
                        nc.tensor.matmul(
                            p_h[:],
                            xqT[:, c, :],
                            W[:, c, h * UH:(h + 1) * UH],
                            start=(c == 0), stop=(c == DC - 1),
                        )
                # split PSUM evacuation with fused per-partition beta
                # scale: h0 on DVE, h1 on ACT (bf16 out)
                osb = opool.tile([P, U], BF16, tag="osb")
                bcol = beta_cols[:, t:t + 1]
                nc.vector.tensor_scalar(
                    osb[:, 0:UH], ps[0][:], bcol, None, op0=ALU.mult
                )
                nc.scalar.activation(
                    osb[:, UH:U], ps[1][:], AF.Copy, scale=bcol
                )
                nc.sync.dma_start(out=out_d[t * P:(t + 1) * P, :], in_=osb[:])
                if t + LOOK < NT:
                    xqTs[t + LOOK] = prep(t + LOOK)

    nc.compile()
    return nc


_CACHE = {}


def _get_nc():
    if "nc" not in _CACHE:
        _CACHE["nc"] = build_nc()
    return _CACHE["nc"]


def make_in_maps(x, k0, k1, k2, a0, a1, a2):
    x = np.ascontiguousarray(x, dtype=np.float32)
    ks = [np.ascontiguousarray(k, dtype=np.float32) for k in (k0, k1, k2)]
    as_ = [np.ascontiguousarray(a, dtype=np.float32).reshape(1, U) for a in (a0, a1, a2)]
    in_maps = []
    for i in range(NCORES):
        shard = np.ascontiguousarray(x[i * NS:(i + 1) * NS])
        in_maps.append({
            "x": shard,
            **{f"k{e}": ks[e] for e in range(E)},
            **{f"a{e}": as_[e] for e in range(E)},
        })
    return in_maps


def run_sharded(x, k0, k1, k2, a0, a1, a2, trace=False, **kw):
    nc = _get_nc()
    in_maps = make_in_maps(x, k0, k1, k2, a0, a1, a2)
    res = run_bass_kernel_spmd(nc, in_maps, list(range(NCORES)), trace=trace, **kw)
    out = np.concatenate(
        [np.asarray(res.results[i]["out"]).astype(np.float32) for i in range(NCORES)],
        axis=0,
    )
    return out, res


def kernel(x, k0, k1, k2, a0, a1, a2):
    out, _ = run_sharded(x, k0, k1, k2, a0, a1, a2, trace=False)
    return out


# revision 11
# speedup vs baseline: 1.4801x; 1.2290x over previous
"""ABCDense (ShiftedSteSign 3-estimator binary dense) Trainium2 kernel.

Math:
    xq   = sign(x)                      [N, D]   (+1 for x >= 0)
    beta = mean(|x|, axis=-1)           [N]
    out  = sum_e (xq @ sign(k_e)) * (beta[:,None] * a_e[None,:])

Folded form used here (column scaling commutes with the GEMM):
    W    = sum_e sign(k_e) * (a_e / D)[None,:]        [D, U]  (bf16)
    out  = sum_abs_x[:,None] * (xq @ W)

Distribution: pure data-parallel over the N=32768 token axis across 8
cores (4096 rows each); k/a replicated (no collectives: any cross-core
sync point adds the cores' launch skew to the measured span).

Queue decoupling (the point of this version): the two HWDGE rings are
split by role -- Sync issues only x loads / k loads / out stores, the
Scalar (ACT) sequencer issues the xbar transposes right after the sign
it depends on (same FIFO, so no cross-engine wait can block the Sync
ring's head).  PSUM evacuation is split DVE(h0)/ACT(h1) so neither
vector FIFO couples next-tile prep to current-tile matmuls.

Per-tile: x f32 -> DVE |x| fused row-sum (beta) -> ACT Sign f32->bf16
natural -> ACT-issued xbar transpose (SBUF->SBUF bf16) -> 16 bf16
matmuls (h-outer: 8 into each PSUM bank) -> split evac with fused
per-partition beta scale -> out DMA.  The PE does nothing but the 512
N=512 matmuls; the +-1 GEMM is exact in bf16.
"""

import numpy as np

import concourse.mybir as mybir
from concourse import bacc, tile
from concourse.bass_utils import run_bass_kernel_spmd

F32 = mybir.dt.float32
BF16 = mybir.dt.bfloat16
AF = mybir.ActivationFunctionType
ALU = mybir.AluOpType

N, D, U, E = 32768, 1024, 1024, 3
NCORES = 8
NS = N // NCORES            # 4096 rows per core
P = 128                     # partitions
DC = D // P                 # 8 d-chunks
NT = NS // P                # 32 n-tiles of 128 tokens
UH = 512                    # u half (one PSUM bank per matmul)
LOOK = 8                    # software pipeline depth (tiles)


def build_nc():
    nc = bacc.Bacc(
        "TRN2",
        target_bir_lowering=False,
        debug=False,
        num_devices=NCORES,
    )

    # --- DRAM parameters (per-core shapes) ---
    x_d = nc.dram_tensor("x", [NS, D], F32, kind="ExternalInput")
    ks = [nc.dram_tensor(f"k{e}", [D, U], F32, kind="ExternalInput") for e in range(E)]
    as_ = [nc.dram_tensor(f"a{e}", [1, U], F32, kind="ExternalInput") for e in range(E)]
    out_d = nc.dram_tensor("out", [NS, U], BF16, kind="ExternalOutput")

    with tile.TileContext(nc) as tc:
        with (
            tc.tile_pool(name="const", bufs=1) as const,
            tc.tile_pool(name="kstage", bufs=2) as kpool,
            tc.tile_pool(name="xin", bufs=LOOK + 1) as xpool,
            tc.tile_pool(name="scr", bufs=3) as scrpool,
            tc.tile_pool(name="xqn", bufs=3) as xqnpool,
            tc.tile_pool(name="xqt", bufs=LOOK + 2) as xqtpool,
            tc.tile_pool(name="osb", bufs=3) as opool,
            tc.tile_pool(name="psM", bufs=8, space="PSUM") as psumM,
        ):
            beta_cols = const.tile([P, NT], F32)

            # a loads first (tiny; the folds need them early)
            a_fs = []
            for e in range(E):
                a_f = const.tile([1, U], F32, tag=f"a_f{e}")
                nc.sync.dma_start(out=a_f[0:1, :], in_=as_[e][:, :])
                a_fs.append(a_f)

            # k loads: one 2MB dma_start per estimator (splits across all
            # 16 SDMA engine slots internally; >=1MiB for full rate)
            kes = []
            for e in range(E):
                ke = kpool.tile([P, DC, U], F32, tag="ke")
                nc.sync.dma_start(
                    out=ke[:],
                    in_=ks[e][:, :].rearrange("(c p) u -> p c u", p=P),
                )
                kes.append(ke)

            a_bcast = []
            for e in range(E):
                a_b = const.tile([1, U], BF16, tag=f"a_b{e}")
                nc.vector.tensor_scalar(
                    a_b[0:1, :], a_fs[e][0:1, :], 1.0 / D, None, op0=ALU.mult
                )
                a_full = const.tile([P, U], BF16, tag=f"a_full{e}")
                nc.gpsimd.partition_broadcast(a_full[:], a_b[0:1, :])
                a_bcast.append(a_full)

            # ---------- W = sum_e sign(k_e) * a_e / D  (bf16, [d-part, c, u]) ----------
            W = const.tile([P, DC, U], BF16)
            for e in range(E):
                ke = kes[e]
                for q in range(4):
                    s_q = kpool.tile([P, 2, U], BF16, tag="se")
                    nc.scalar.activation(
                        s_q[:], ke[:, q * 2:(q + 1) * 2, :], AF.Sign,
                    )
                    for cc in range(2):
                        c = q * 2 + cc
                        if e == 0:
                            nc.vector.tensor_tensor(
                                W[:, c, :], s_q[:, cc, :], a_bcast[e][:],
                                op=ALU.mult
                            )
                        else:
                            tmp = kpool.tile([P, U], BF16, tag="tmp")
                            nc.vector.tensor_tensor(
                                tmp[:], s_q[:, cc, :], a_bcast[e][:],
                                op=ALU.mult
                            )
                            nc.vector.tensor_tensor(
                                W[:, c, :], W[:, c, :], tmp[:], op=ALU.add
                            )

            # ---------- software-pipelined main loop ----------
            def prep(t):
                x_t = xpool.tile([P, D], F32, tag="xt")
                nc.sync.dma_start(out=x_t[:], in_=x_d[t * P:(t + 1) * P, :])
                # beta: |x|=max(-x,x) with fused row-sum on DVE (raw sum;
                # the 1/D of beta is folded into W)
                scratch = scrpool.tile([P, D], F32, tag="scratch")
                nc.vector.scalar_tensor_tensor(
                    scratch[:], x_t[:], -1.0, x_t[:],
                    op0=ALU.mult, op1=ALU.max,
                    accum_out=beta_cols[:, t:t + 1],
                )
                # sign (f32 -> bf16, natural) on ACT, then the xbar
                # transpose issued from the ACT sequencer (same FIFO)
                xq_n = xqnpool.tile([P, D], BF16, tag="xqn")
                nc.scalar.activation(xq_n[:], x_t[:], AF.Sign)
                xqT = xqtpool.tile([P, DC, P], BF16, tag="xqT")
                nc.sync.dma_start_transpose(xqT[:], xq_n[:])
                return xqT

            xqTs = {}
            for t in range(LOOK):
                xqTs[t] = prep(t)

            for t in range(NT):
                xqT = xqTs.pop(t)
                ps = []
                for h in range(2):
                    p_h = psumM.tile([P, UH], F32, tag="ps")
                    ps.append(p_h)
                    for c in range(DC):
                        nc.tensor.matmul(
                            p_h[:],
                            xqT[:, c, :],
                            W[:, c, h * UH:(h + 1) * UH],
                            start=(c == 0), stop=(c == DC - 1),
                        )
                # split PSUM evacuation with fused per-partition beta
                # scale: h0 on DVE, h1 on ACT (bf16 out)
                osb = opool.tile([P, U], BF16, tag="osb")
                bcol = beta_cols[:, t:t + 1]
                nc.vector.tensor_scalar(
                    osb[:, 0:UH], ps[0][:], bcol, None, op0=ALU.mult
                )
                nc.scalar.activation(
                    osb[:, UH:U], ps[1][:], AF.Copy, scale=bcol
                )
                # out store via gpsimd (SWDGE): its completion counts on the
                # DMASW sem lanes, keeping the compute-gated store off the 8
                # shared DMAHW lanes that the transposes/x-loads wait on
                # (round-robin lane aliasing would otherwise chain every MM
                # block to the previous tile's evacuation)
                nc.gpsimd.dma_start(out=out_d[t * P:(t + 1) * P, :], in_=osb[:])
                if t + LOOK < NT:
                    xqTs[t + LOOK] = prep(t + LOOK)

    nc.compile()
    return nc


_CACHE = {}


def _get_nc():
    if "nc" not in _CACHE:
        _CACHE["nc"] = build_nc()
    return _CACHE["nc"]


def make_in_maps(x, k0, k1, k2, a0, a1, a2):
    x = np.ascontiguousarray(x, dtype=np.float32)
    ks = [np.ascontiguousarray(k, dtype=np.float32) for k in (k0, k1, k2)]
    as_ = [np.ascontiguousarray(a, dtype=np.float32).reshape(1, U) for a in (a0, a1, a2)]
    in_maps = []
    for i in range(NCORES):
        shard = np.ascontiguousarray(x[i * NS:(i + 1) * NS])
        in_maps.append({
            "x": shard,
            **{f"k{e}": ks[e] for e in range(E)},
            **{f"a{e}": as_[e] for e in range(E)},
        })
    return in_maps


def run_sharded(x, k0, k1, k2, a0, a1, a2, trace=False, **kw):
    nc = _get_nc()
    in_maps = make_in_maps(x, k0, k1, k2, a0, a1, a2)
    res = run_bass_kernel_spmd(nc, in_maps, list(range(NCORES)), trace=trace, **kw)
    out = np.concatenate(
        [np.asarray(res.results[i]["out"]).astype(np.float32) for i in range(NCORES)],
        axis=0,
    )
    return out, res


def kernel(x, k0, k1, k2, a0, a1, a2):
    out, _ = run_sharded(x, k0, k1, k2, a0, a1, a2, trace=False)
    return out
